# revision 10
# baseline (speedup 1.0000x reference)
"""CARAFE kernel for 8 TRN2 NeuronCores (Bass/Tile, SPMD).

Math (see reference):
  k0   = w_comp @ x + b_comp                 (64, 32, 32)      1x1 conv
  kc   = w_ker (*) k0 + b_ker                (102400, 32, 32)  3x3 conv, pad 1
  k    = softmax(kc.reshape(4, 25600, H, W), axis=1)
  ksum = k.sum(axis=1)                       (4, 32, 32)       == S/S (==1+eps)
  out  = (x[:, :, None] * ksum[:, None]).reshape(1, 256, 64, 64)

Sharding: tensor-parallel over the 102400 conv output channels, 12800 per
core. Each softmax group (25600 chans) spans cores (2s, 2s+1); group sums
are combined with a pairwise AllReduce. Core 2s+h computes the final
output for scale-group s, channel half h (128 of 256 x-channels).

Device layout choices:
  * The 3x3 conv is a matmul with contraction K = 64*9 (+1 bias row) = 577,
    M = 1024 pixels (PSUM partitions), N = 12800 channels (free dim).
    Channels on the free dim let ScalarE's Exp produce per-pixel partial
    softmax sums via accum_out for free.
  * b_ker is folded into the matmul as an extra contraction row whose
    im2col entry is 1.0.
  * Matmul operands are bitcast to float32r (full PE rate at N=512).
"""

import numpy as np

import concourse.bass as bass
import concourse.mybir as mybir
import concourse.tile as tile
from concourse import bacc
from concourse.bass_utils import run_bass_kernel_spmd

F32 = mybir.dt.float32
F32R = mybir.dt.float32r
AF = mybir.ActivationFunctionType

# Problem constants
C, H, W = 256, 32, 32
CH = 64                   # compressed channels
NPIX = H * W              # 1024
OC_TOTAL = 102400
NCORES = 8
OC = OC_TOTAL // NCORES   # 12800 channels per core
KDIM = CH * 9             # 576
KTOT = KDIM + 1           # +1 bias row
NK = 5                    # contraction k-tiles (4x128 + 65)
NT = OC // 512            # 25 channel tiles of 512
MT = NPIX // 128          # 8 pixel tiles of 128
CHALF = C // 2            # 128 x-channels per core


def build():
    nc = bacc.Bacc("TRN2", target_bir_lowering=False, debug=False,
                   num_devices=NCORES)

    xf = nc.dram_tensor("xf", [C, NPIX], F32R, kind="ExternalInput")
    xt = nc.dram_tensor("xt", [NPIX, CHALF], F32, kind="ExternalInput")
    wc = nc.dram_tensor("wc", [C, CH], F32R, kind="ExternalInput")
    bc = nc.dram_tensor("bc", [CH, 1], F32, kind="ExternalInput")
    wk = nc.dram_tensor("wk", [KTOT, OC], F32R, kind="ExternalInput")
    ones = nc.dram_tensor("ones", [1, NPIX], F32R, kind="ExternalInput")
    out = nc.dram_tensor("out", [NPIX, CHALF], F32, kind="ExternalOutput")
    sdbg = nc.dram_tensor("sdbg", [128, MT], F32, kind="ExternalOutput")

    with tile.TileContext(nc) as tc:
        with (
            tc.tile_pool(name="const", bufs=1) as const,
            tc.tile_pool(name="wpool", bufs=4) as wpool,
            tc.tile_pool(name="ppool", bufs=8, space="PSUM") as ppool,
            tc.tile_pool(name="epool", bufs=4) as epool,
            tc.tile_pool(name="dram", bufs=1, space="DRAM") as dram,
        ):
            # ---- constants / staging ----
            x_sb = const.tile([128, 2, NPIX], F32R)
            nc.sync.dma_start(x_sb[:], xf.ap().rearrange("(k p) n -> p k n", p=128))
            wc_sb = const.tile([128, 2, CH], F32R)
            nc.sync.dma_start(wc_sb[:], wc.ap().rearrange("(k p) m -> p k m", p=128))
            bc_sb = const.tile([CH, 1], F32)
            nc.sync.dma_start(bc_sb[:], bc.ap())
            xt_sb = const.tile([128, MT, CHALF], F32)
            nc.sync.dma_start(xt_sb[:], xt.ap().rearrange("(m p) c -> p m c", p=128))

            # ---- compress conv: k0 = w_comp @ x + b_comp ----
            k0p = const.tile([CH, H + 2, W + 2], F32R)   # padded (34x34)
            nc.vector.memset(k0p[:].bitcast(F32), 0.0)
            for nh in range(2):
                cps = ppool.tile([128, 512], F32, tag="ps", name=f"cps_{nh}")
                for kt in range(2):
                    nc.tensor.matmul(
                        cps[0:CH, :],
                        lhsT=wc_sb[:, kt, :],
                        rhs=x_sb[:, kt, nh * 512:(nh + 1) * 512],
                        start=(kt == 0), stop=(kt == 1),
                    )
                # evict into padded interior (16 image rows per half), + bias
                nc.scalar.activation(
                    k0p[:, 1 + nh * 16:1 + (nh + 1) * 16, 1:W + 1],
                    cps[0:CH, :].rearrange("p (a b) -> p a b", a=16),
                    AF.Identity, bias=bc_sb[:],
                )

            # ---- im2col: E[kk, pix], kk = tap*64 + ci, row 576 = ones ----
            E = const.tile([128, NK, NPIX], F32R)
            for t in range(9):
                dh, dw = t // 3, t % 3
                kt, half = t // 2, t % 2
                nc.sync.dma_start(
                    E[half * 64:(half + 1) * 64, kt, :],
                    k0p[:, dh:dh + H, dw:dw + W],
                )
            nc.sync.dma_start(E[64:65, 4, :], ones.ap())

            # ---- big conv + exp + per-pixel partial sums ----
            spart = const.tile([128, MT * NT], F32)     # (pix, m*NT+n)
            for n in range(NT):
                wt = wpool.tile([128, NK, 512], F32R, tag="wt")
                for kt in range(4):
                    nc.sync.dma_start(
                        wt[:, kt, :],
                        wk.ap()[kt * 128:(kt + 1) * 128, n * 512:(n + 1) * 512],
                    )
                nc.sync.dma_start(
                    wt[0:65, 4, :],
                    wk.ap()[512:KTOT, n * 512:(n + 1) * 512],
                )
                for mg in range(2):
                    pts = [
                        ppool.tile([128, 512], F32, tag="ps", name=f"ps_{n}_{mg}_{i}")
                        for i in range(4)
                    ]
                    for kt in range(NK):
                        kk = 128 if kt < 4 else 65
                        for mi in range(4):
                            m = mg * 4 + mi
                            nc.tensor.matmul(
                                pts[mi][:],
                                lhsT=E[0:kk, kt, m * 128:(m + 1) * 128],
                                rhs=wt[0:kk, kt, :],
                                start=(kt == 0), stop=(kt == NK - 1),
                            )
                    for mi in range(4):
                        m = mg * 4 + mi
                        et = epool.tile([128, 512], F32, tag="et")
                        nc.scalar.activation(
                            et[:], pts[mi][:], AF.Exp,
                            accum_out=spart[:, m * NT + n:m * NT + n + 1],
                        )

            # ---- per-core softmax sums -> pairwise AllReduce ----
            S = const.tile([128, MT], F32)
            for m in range(MT):
                nc.vector.tensor_reduce(
                    S[:, m:m + 1], spart[:, m * NT:(m + 1) * NT],
                    axis=mybir.AxisListType.X, op=mybir.AluOpType.add,
                )
            s_in = dram.tile([128, MT], F32)
            s_out = dram.tile([128, MT], F32)
            nc.sync.dma_start(s_in[:], S[:])
            nc.gpsimd.collective_compute(
                "AllReduce", mybir.AluOpType.add,
                replica_groups=[[0, 1], [2, 3], [4, 5], [6, 7]],
                ins=[s_in[:]], outs=[s_out[:]],
            )
            Sg = const.tile([128, MT], F32)
            nc.sync.dma_start(Sg[:], s_out[:])
            nc.sync.dma_start(sdbg.ap(), Sg[:])

            # ---- ksum = S/S ; out = x^T * ksum ----
            rec = const.tile([128, MT], F32)
            nc.vector.reciprocal(rec[:], Sg[:])
            ks = const.tile([128, MT], F32)
            nc.vector.tensor_mul(ks[:], Sg[:], rec[:])
            ot = const.tile([128, MT, CHALF], F32)
            for m in range(MT):
                nc.vector.tensor_scalar_mul(
                    ot[:, m, :], xt_sb[:, m, :], ks[:, m:m + 1],
                )
            nc.sync.dma_start(out.ap().rearrange("(m p) c -> p m c", p=128), ot[:])

    nc.compile()
    return nc


_NC = None


def _get_nc():
    global _NC
    if _NC is None:
        _NC = build()
    return _NC


def prep_inputs(x, w_comp, b_comp, w_ker, b_ker):
    xf = np.ascontiguousarray(x.reshape(C, NPIX), dtype=np.float32)
    xt_full = np.ascontiguousarray(xf.T)                      # (1024, 256)
    wcT = np.ascontiguousarray(w_comp.reshape(CH, C).T)       # (256, 64)
    bcr = np.ascontiguousarray(b_comp.reshape(CH, 1), dtype=np.float32)
    wt = np.empty((KTOT, OC_TOTAL), dtype=np.float32)
    wt[:KDIM] = w_ker.reshape(OC_TOTAL, CH, 9).transpose(2, 1, 0).reshape(KDIM, OC_TOTAL)
    wt[KDIM] = b_ker
    in_maps = []
    for core in range(NCORES):
        h = core % 2
        in_maps.append({
            "xf": xf,
            "xt": np.ascontiguousarray(xt_full[:, h * CHALF:(h + 1) * CHALF]),
            "wc": wcT,
            "bc": bcr,
            "wk": np.ascontiguousarray(wt[:, core * OC:(core + 1) * OC]),
            "ones": np.ones((1, NPIX), dtype=np.float32),
        })
    return in_maps


def assemble(results, x):
    full = np.empty((C, 2 * H, 2 * W), dtype=np.float32)
    for core in range(NCORES):
        s, h = core // 2, core % 2
        blk = results[core]["out"]                            # (1024, 128)
        full[h * CHALF:(h + 1) * CHALF, s * 16:(s + 1) * 16, :] = (
            blk.T.reshape(CHALF, 16, 64)
        )
    return full.reshape(1, C, 2 * H, 2 * W)


def run(in_maps, trace=False, **kw):
    nc = _get_nc()
    return run_bass_kernel_spmd(nc, in_maps, list(range(NCORES)), trace=trace, **kw)


def kernel(x, w_comp, b_comp, w_ker, b_ker):
    in_maps = prep_inputs(x, w_comp, b_comp, w_ker, b_ker)
    res = run(in_maps)
    return assemble(res.results, x)


# revision 28
# speedup vs baseline: 1.0915x; 1.0915x over previous
"""CARAFE kernel for 8 TRN2 NeuronCores (Bass/Tile, SPMD).

Math (see reference):
  k0   = w_comp @ x + b_comp                 (64, 32, 32)      1x1 conv
  kc   = w_ker (*) k0 + b_ker                (102400, 32, 32)  3x3 conv, pad 1
  k    = softmax(kc.reshape(4, 25600, H, W), axis=1)
  ksum = k.sum(axis=1)                       (4, 32, 32)       == S/S (==1+eps)
  out  = (x[:, :, None] * ksum[:, None]).reshape(1, 256, 64, 64)

Sharding: tensor-parallel over the 102400 conv output channels, 12800 per
core. Each softmax group (25600 chans) spans cores (2s, 2s+1); group sums
are combined with a pairwise AllReduce. Core 2s+h computes the final
output for scale-group s, channel half h (128 of 256 x-channels).

Device layout choices:
  * The 3x3 conv is a matmul with contraction K = 64*9 (+1 bias row) = 577,
    M = 1024 pixels (PSUM partitions), N = 12800 channels (free dim).
    Channels on the free dim let ScalarE's Exp produce per-pixel partial
    softmax sums via accum_out for free.
  * No materialized im2col. The 9 conv taps are paired so each pair's two
    window offsets differ by a constant flat delta (+1 within an image row,
    +34 = one padded row). Three 128-partition copies of the padded
    compressed image serve as matmul lhsT directly via sliced window APs:
      T1 = [A; A<<1]  for tap pairs (0,1) (3,4) (6,7)
      T2 = [A; A<<34] for tap pair  (2,5)
      T3 = [A; ones]  for tap 8 + the bias row (K=65)
    The shifted upper halves are single contiguous SBUF->SBUF DMAs.
  * b_ker is folded into the matmul as the extra all-ones contraction row.
  * Conv compute in bf16: softmax sums are divided by themselves (ksum==1
    in exact arithmetic), so conv precision does not reach the output.
  * W is zero-padded to 640 contraction rows and blocked per (core, n-tile)
    on the host so each weight tile loads as one DMA of 128 partitions x 5KB
    contiguous (near-peak HBM bandwidth).
"""

import numpy as np

import concourse.bass as bass
import concourse.mybir as mybir
import concourse.tile as tile
from concourse import bacc
from concourse.bass_utils import run_bass_kernel_spmd

F32 = mybir.dt.float32
BF16 = mybir.dt.bfloat16
AF = mybir.ActivationFunctionType

# Problem constants
C, H, W = 256, 32, 32
CH = 64                   # compressed channels
NPIX = H * W              # 1024
OC_TOTAL = 102400
NCORES = 8
OC = OC_TOTAL // NCORES   # 12800 channels per core
KDIM = CH * 9             # 576
NK = 5                    # contraction k-tiles (4x128 + 65)
WROWS = NK * 128          # host-padded W rows (640)
NT = OC // 512            # 25 channel tiles of 512
MT = NPIX // 128          # 8 pixel tiles of 128
CHALF = C // 2            # 128 x-channels per core
PADW = W + 2              # 34

# tap pairing: k-tile kt holds taps (LOWTAP[kt], LOWTAP[kt]+delta) on
# partitions [0:64) and [64:128); T3 holds tap 8 + the bias ones row.
# tap t = (dh, dw) = (t // 3, t % 3), flat offset dh*34 + dw.
LOWTAP = [0, 3, 6, 2, 8]                  # kt -> low tap
TAPORDER = [0, 1, 3, 4, 6, 7, 2, 5, 8]    # W row grouping (matches pairs)


def build():
    nc = bacc.Bacc("TRN2", target_bir_lowering=False, debug=False,
                   num_devices=NCORES)

    xf = nc.dram_tensor("xf", [C, NPIX], BF16, kind="ExternalInput")
    xt = nc.dram_tensor("xt", [NPIX, CHALF], F32, kind="ExternalInput")
    wc = nc.dram_tensor("wc", [C, CH], BF16, kind="ExternalInput")
    bc = nc.dram_tensor("bc", [CH, 1], F32, kind="ExternalInput")
    wk = nc.dram_tensor("wk", [NT, 128, NK, 512], BF16, kind="ExternalInput")
    out = nc.dram_tensor("out", [NPIX, CHALF], F32, kind="ExternalOutput")
    sdbg = nc.dram_tensor("sdbg", [128, MT], F32, kind="ExternalOutput")

    with tile.TileContext(nc) as tc:
        with (
            tc.tile_pool(name="const", bufs=1) as const,
            tc.tile_pool(name="wpool", bufs=8) as wpool,
            tc.tile_pool(name="ppool", bufs=8, space="PSUM") as ppool,
            tc.tile_pool(name="epool", bufs=4) as epool,
            tc.tile_pool(name="dram", bufs=1, space="DRAM") as dram,
        ):
            def load_wt(n):
                # W is host-blocked per n-tile: 128 partitions x 5KB
                # contiguous, so one DMA runs at near-peak bandwidth
                wt = wpool.tile([128, NK, 512], BF16, tag="wt", name=f"wt_{n}")
                nc.sync.dma_start(wt[:], wk.ap()[n])
                return wt

            # ---- constants / staging (W n=0 hoisted ahead) ----
            wc_sb = const.tile([128, 2, CH], BF16)
            nc.sync.dma_start(wc_sb[:], wc.ap().rearrange("(k p) m -> p k m", p=128))
            bc_sb = const.tile([CH, 1], F32)
            nc.sync.dma_start(bc_sb[:], bc.ap())
            x_r = xf.ap().rearrange("(k p) n -> p k n", p=128)
            x_sb = const.tile([128, 2, NPIX], BF16)
            nc.sync.dma_start(x_sb[:, 0, :], x_r[:, 0, :])
            nc.sync.dma_start(x_sb[:, 1, :], x_r[:, 1, :])
            wts = {0: load_wt(0)}
            xt_sb = const.tile([128, MT, CHALF], F32)

            # padded-image composite tiles (halo zeros via memset; the upper
            # halves of T1/T2 are fully overwritten by the shift DMAs)
            T1 = const.tile([128, PADW, PADW], BF16)
            T2 = const.tile([128, PADW, PADW], BF16)
            T3 = const.tile([128, PADW, PADW], BF16)
            nc.vector.memset(T1[:], 0.0)
            nc.vector.memset(T3[0:64], 0.0)
            nc.vector.memset(T3[64:65], 1.0)
            nc.gpsimd.memset(T2[:], 0.0)

            # ---- compress conv: k0 = w_comp @ x + b_comp ----
            for nh in range(2):
                cps = ppool.tile([128, 512], F32, tag="ps", name=f"cps_{nh}")
                for kt in range(2):
                    nc.tensor.matmul(
                        cps[0:CH, :],
                        lhsT=wc_sb[:, kt, :],
                        rhs=x_sb[:, kt, nh * 512:(nh + 1) * 512],
                        start=(kt == 0), stop=(kt == 1),
                    )
                # evict (16 image rows per half) into T1's interior, + bias
                nc.scalar.activation(
                    T1[0:CH, 1 + nh * 16:1 + (nh + 1) * 16, 1:W + 1],
                    cps[0:CH, :].rearrange("p (a b) -> p a b", a=16),
                    AF.Identity, bias=bc_sb[:],
                )
            # replicate A into T2/T3 lower halves (partition-aligned fast DMAs)
            nc.sync.dma_start(T2[0:64, 1:H + 1, :], T1[0:64, 1:H + 1, :])
            nc.gpsimd.dma_start(T3[0:64, 1:H + 1, :], T1[0:64, 1:H + 1, :])

            # shifted upper halves: one contiguous SBUF->SBUF DMA each
            flat1 = T1[:].rearrange("p a b -> p (a b)")
            nc.sync.dma_start(flat1[64:128, 0:PADW * PADW - 1],
                              flat1[0:64, 1:PADW * PADW])
            flat2 = T2[:].rearrange("p a b -> p (a b)")
            nc.sync.dma_start(flat2[64:128, 0:PADW * PADW - PADW],
                              flat2[0:64, PADW:PADW * PADW])

            def lhsT_ap(kt, m, j):
                # one 32-pixel image row (single free dim) for col-tile j
                T = (T1, T1, T1, T2, T3)[kt]
                dh, dw = LOWTAP[kt] // 3, LOWTAP[kt] % 3
                kk = 128 if kt < 4 else 65
                r = dh + 4 * m + j
                return T[0:kk, r:r + 1, dw:dw + W]

            # ---- big conv + exp + per-pixel partial sums ----
            spart = const.tile([128, MT * NT], F32)     # (pix, m*NT+n)
            for n in range(NT):
                wt = wts.pop(n) if n in wts else load_wt(n)
                if n == 2:
                    # x^T load (only needed by the tail) off the startup path
                    nc.gpsimd.dma_start(
                        xt_sb[:], xt.ap().rearrange("(m p) c -> p m c", p=128))
                for mg in range(2):
                    pts = [
                        ppool.tile([128, 512], F32, tag="ps", name=f"ps_{n}_{mg}_{i}")
                        for i in range(4)
                    ]
                    for kt in range(NK):
                        kk = 128 if kt < 4 else 65
                        for mi in range(4):
                            m = mg * 4 + mi
                            for j in range(4):
                                nc.tensor.matmul(
                                    pts[mi][32 * j:32 * (j + 1), :],
                                    lhsT=lhsT_ap(kt, m, j),
                                    rhs=wt[0:kk, kt, :],
                                    start=(kt == 0), stop=(kt == NK - 1),
                                    tile_position=(0, 32 * j),
                                )
                    for mi in range(4):
                        m = mg * 4 + mi
                        et = epool.tile([128, 512], F32, tag="et")
                        nc.scalar.activation(
                            et[:], pts[mi][:], AF.Exp,
                            accum_out=spart[:, m * NT + n:m * NT + n + 1],
                        )

            # ---- per-core softmax sums -> pairwise AllReduce ----
            S = const.tile([128, MT], F32)
            for m in range(MT):
                nc.vector.tensor_reduce(
                    S[:, m:m + 1], spart[:, m * NT:(m + 1) * NT],
                    axis=mybir.AxisListType.X, op=mybir.AluOpType.add,
                )
            s_in = dram.tile([128, MT], F32)
            s_out = dram.tile([128, MT], F32)
            nc.sync.dma_start(s_in[:], S[:])
            nc.gpsimd.collective_compute(
                "AllReduce", mybir.AluOpType.add,
                replica_groups=[[0, 1], [2, 3], [4, 5], [6, 7]],
                ins=[s_in[:]], outs=[s_out[:]],
            )
            Sg = const.tile([128, MT], F32)
            nc.sync.dma_start(Sg[:], s_out[:])
            nc.sync.dma_start(sdbg.ap(), Sg[:])

            # ---- ksum = S/S ; out = x^T * ksum ----
            rec = const.tile([128, MT], F32)
            nc.vector.reciprocal(rec[:], Sg[:])
            ks = const.tile([128, MT], F32)
            nc.vector.tensor_mul(ks[:], Sg[:], rec[:])
            ot = const.tile([128, MT, CHALF], F32)
            for m in range(MT):
                nc.vector.tensor_scalar_mul(
                    ot[:, m, :], xt_sb[:, m, :], ks[:, m:m + 1],
                )
            nc.sync.dma_start(out.ap().rearrange("(m p) c -> p m c", p=128), ot[:])

    nc.compile()
    return nc


_NC = None


def _get_nc():
    global _NC
    if _NC is None:
        _NC = build()
    return _NC


def prep_inputs(x, w_comp, b_comp, w_ker, b_ker):
    import ml_dtypes
    x = np.asarray(x, dtype=np.float32)
    w_comp = np.asarray(w_comp, dtype=np.float32)
    b_comp = np.asarray(b_comp, dtype=np.float32)
    w_ker = np.asarray(w_ker, dtype=np.float32)
    b_ker = np.asarray(b_ker, dtype=np.float32)
    xf = np.ascontiguousarray(x.reshape(C, NPIX)).astype(ml_dtypes.bfloat16)
    xt_full = np.ascontiguousarray(x.reshape(C, NPIX).astype(np.float32).T)
    wcT = np.ascontiguousarray(w_comp.reshape(CH, C).T).astype(ml_dtypes.bfloat16)
    bcr = np.ascontiguousarray(b_comp.reshape(CH, 1), dtype=np.float32)
    wt = np.zeros((WROWS, OC_TOTAL), dtype=ml_dtypes.bfloat16)
    w9 = w_ker.reshape(OC_TOTAL, CH, 9)[:, :, TAPORDER]     # (O, 64, 9 slots)
    wt[:KDIM] = w9.transpose(2, 1, 0).reshape(KDIM, OC_TOTAL)
    wt[KDIM] = b_ker                                        # row 576 = bias
    # per-core, per-n-tile contiguous blocks: (NT, 128, NK, 512)
    wtb = wt.reshape(NK, 128, NCORES, NT, 512).transpose(2, 3, 1, 0, 4)
    in_maps = []
    for core in range(NCORES):
        h = core % 2
        in_maps.append({
            "xf": xf,
            "xt": np.ascontiguousarray(xt_full[:, h * CHALF:(h + 1) * CHALF]),
            "wc": wcT,
            "bc": bcr,
            "wk": np.ascontiguousarray(wtb[core]),
        })
    return in_maps


def assemble(results, x):
    full = np.empty((C, 2 * H, 2 * W), dtype=np.float32)
    for core in range(NCORES):
        s, h = core // 2, core % 2
        blk = results[core]["out"]                            # (1024, 128)
        full[h * CHALF:(h + 1) * CHALF, s * 16:(s + 1) * 16, :] = (
            blk.T.reshape(CHALF, 16, 64)
        )
    return full.reshape(1, C, 2 * H, 2 * W)


def run(in_maps, trace=False, **kw):
    nc = _get_nc()
    return run_bass_kernel_spmd(nc, in_maps, list(range(NCORES)), trace=trace, **kw)


def kernel(x, w_comp, b_comp, w_ker, b_ker):
    in_maps = prep_inputs(x, w_comp, b_comp, w_ker, b_ker)
    res = run(in_maps)
    return assemble(res.results, x)
